# revision 12
# baseline (speedup 1.0000x reference)
"""Trainium2 Bass kernel for nn_FLB_Attention_Layer (gated fusion + additive
attention over 3 tokens + output projection, with residuals).

Strategy: pure data-parallel over batch B=4096 across 8 NeuronCores
(512 samples/core, weights replicated). Inside each core:

- All activations are kept FEATURE-MAJOR in SBUF: [128 part = feature%128,
  k-tile = feature//128, token, batch]. Matmuls contract features on the
  partition dim (lhsT = W.T column block, rhs = activations).
- Weights are loaded row-contiguous [128 out-rows, 2048] and transposed
  on-chip via TensorE transpose-mode into W.T column blocks.
- Matmuls run in float32r (fp32 data, ~tf32 accuracy, bf16-rate at N>=512).
- Additive attention per head h (head dim 128 = one partition tile):
  T = tanh(q_i + k_j) built by DVE+ACT (bf16), scores via PE matmuls
  (lhsT = T slice, rhs = v_a column) -> batch-major scores [128b, 9],
  softmax batch-major on DVE/ACT, weights transposed back via PE and
  broadcast across partitions with a constant row-select matmul (SEL),
  weighted sum of v on DVE.
- Attention output (feature-major) roundtrips through a DRAM scratch,
  reusing the token SBUF space for the W_o contraction.
- Residual add done batch-major right before the final store.
"""

import numpy as np

P = 128
D = 2048
H = 16
DH = 128
KT = D // P  # 16 k-tiles
B = 4096
N_CORES = 8
B_C = B // N_CORES  # 512 per core

_compiled = {}


def _build(b_c=B_C, d=D, h=H):
    import concourse.bass as bass
    import concourse.mybir as mybir
    import concourse.tile as tile
    from contextlib import ExitStack
    from concourse import bacc
    from concourse.masks import make_identity

    f32 = mybir.dt.float32
    f32r = mybir.dt.float32r
    bf16 = mybir.dt.bfloat16
    AF = mybir.ActivationFunctionType

    kt = d // P
    nh = h
    nb = b_c // P  # batch tiles

    nc = bacc.Bacc(None, target_bir_lowering=False, debug=False)

    # ---- params ----
    f16 = mybir.dt.float16
    xs = nc.declare_dram_parameter("x", [b_c, d], f16, isOutput=False)
    ls = nc.declare_dram_parameter("lat", [b_c, d], f16, isOutput=False)
    fs = nc.declare_dram_parameter("fdbk", [b_c, d], f16, isOutput=False)
    WgL = nc.declare_dram_parameter("WgL", [d, d], f16, isOutput=False)
    WgX = nc.declare_dram_parameter("WgX", [d, d], f16, isOutput=False)
    Wq = nc.declare_dram_parameter("Wq", [d, d], f16, isOutput=False)
    Wk = nc.declare_dram_parameter("Wk", [d, d], f16, isOutput=False)
    Wv = nc.declare_dram_parameter("Wv", [d, d], f16, isOutput=False)
    Wo = nc.declare_dram_parameter("Wo", [d, d], f16, isOutput=False)
    bgLT = nc.declare_dram_parameter("bgLT", [P, kt], f32, isOutput=False)
    bgXT = nc.declare_dram_parameter("bgXT", [P, kt], f32, isOutput=False)
    vaT = nc.declare_dram_parameter("vaT", [DH, nh], f32, isOutput=False)

    outs = [
        nc.declare_dram_parameter(f"o{t}", [b_c, d], f32, isOutput=True)
        for t in range(3)
    ]
    tok_in = [xs, ls, fs]

    with tile.TileContext(nc) as tc:
        with ExitStack() as ctx:
            const = ctx.enter_context(tc.tile_pool(name="const", bufs=1))
            ptok = ctx.enter_context(tc.tile_pool(name="ptok", bufs=1))
            pbig = ctx.enter_context(tc.tile_pool(name="pbig", bufs=2))
            pwT = ctx.enter_context(tc.tile_pool(name="pwT", bufs=4))
            pact = ctx.enter_context(tc.tile_pool(name="pact", bufs=2))
            pact1 = ctx.enter_context(tc.tile_pool(name="pact1", bufs=2))
            pvh = ctx.enter_context(tc.tile_pool(name="pvh", bufs=2))
            psm = ctx.enter_context(tc.tile_pool(name="psm", bufs=2))
            dram = ctx.enter_context(tc.tile_pool(name="dram", bufs=1, space="DRAM"))
            ps_mm = ctx.enter_context(tc.tile_pool(name="ps_mm", bufs=6, space="PSUM"))
            ps_tp = ctx.enter_context(tc.tile_pool(name="ps_tp", bufs=2, space="PSUM"))

            ident = const.tile([P, P], f32)
            make_identity(nc, ident)
            bgl_t = const.tile([P, kt], f32)
            bgx_t = const.tile([P, kt], f32)
            nc.sync.dma_start(bgl_t[:], bgLT[:])
            nc.sync.dma_start(bgx_t[:], bgXT[:])
            va_f = const.tile([DH, nh], f32)
            nc.sync.dma_start(va_f[:], vaT[:])
            ones = const.tile([P, P], f16)
            nc.any.memset(ones[:], 1.0)

            # tokT: feature-major tokens [p, k, tok, b]; later reused for attT
            tokT = ptok.tile([P, 3, kt, b_c], f16)

            def transpose_block(src_ap, dst_ap):
                """PE-transpose a [128, 128] block; evict (cast) on DVE."""
                tp = ps_tp.tile([P, P], f32, tag="tp")
                nc.tensor.transpose(tp[:], src_ap, ident[:])
                nc.vector.tensor_copy(dst_ap, tp[:])

            # ---- phase 1: XBAR-transposed token loads (feature-major) ----
            for t in (2, 1, 0):
                nc.sync.dma_start_transpose(tokT[:, t, :, :], tok_in[t][:])

            def load_wT(Wt, row_tile):
                """XBAR-transposed load of fp16 weight rows [128, d] into the
                W.T block [128 (in-feature part), kt, 128 (out cols)]."""
                wT = pwT.tile([P, kt, P], f16, tag="wT")
                nc.sync.dma_start_transpose(
                    wT[:], Wt[row_tile * P : (row_tile + 1) * P, :]
                )
                return wT

            # ---- phases 2+3: gated fusion ----
            # G_L = sigmoid(fdbk @ WgL.T + bgL); lat *= G_L
            # G_X = sigmoid(lat' @ WgX.T + bgX); x *= G_X
            for stage, (Wg, bg_t, src_tok, dst_tok) in enumerate(
                [(WgL, bgl_t, 2, 1), (WgX, bgx_t, 1, 0)]
            ):
                for ot in range(kt):
                    wT = load_wT(Wg, ot)
                    pg = ps_mm.tile([P, b_c], f32, tag="mm")
                    for k in range(kt):
                        nc.tensor.matmul(
                            pg[:],
                            wT[:, k, :],
                            tokT[:, src_tok, k, :],
                            start=(k == 0),
                            stop=(k == kt - 1),
                        )
                    gate = pact.tile([P, b_c], f32, tag="gate")
                    nc.scalar.activation(
                        gate[:], pg[:], AF.Sigmoid, bias=bg_t[:, ot : ot + 1]
                    )
                    nc.vector.tensor_mul(
                        tokT[:, dst_tok, ot, :],
                        tokT[:, dst_tok, ot, :],
                        gate[:],
                    )

            # ---- phase 4: per-head QKV + additive attention ----
            attD = [dram.tile([P, 3, b_c], f16, name=f"attD{i}") for i in range(nh)]
            for hh in range(nh):
                qkv_sb = []
                for Wp in (Wq, Wk, Wv):
                    wT = load_wT(Wp, hh)
                    pool_p = pvh if len(qkv_sb) == 2 else pact1
                    dst = pool_p.tile([P, 3, b_c], f16, tag=f"p{len(qkv_sb)}")
                    pps = [ps_mm.tile([P, b_c], f32, tag="mm", name=f"pp{t}") for t in range(3)]
                    for k in range(kt):
                        for t in range(3):
                            nc.tensor.matmul(
                                pps[t][:],
                                wT[:, k, :],
                                tokT[:, t, k, :],
                                start=(k == 0),
                                stop=(k == kt - 1),
                            )
                    for t in range(3):
                        nc.any.tensor_copy(dst[:, t, :], pps[t][:])
                    qkv_sb.append(dst)
                qh, kh, vh = qkv_sb

                # T = tanh(q_i + k_j), bf16 [p, ij, b]
                Tt = pact1.tile([P, 9, b_c], f16, tag="Tt")
                for i in range(3):
                    for j in range(3):
                        pre = pact.tile([P, b_c], f16, tag="Tpre")
                        nc.vector.tensor_add(pre[:], qh[:, i, :], kh[:, j, :])
                        nc.scalar.activation(Tt[:, 3 * i + j, :], pre[:], AF.Tanh)

                # scores: tva = Tt * va_h (per-partition scalar), then
                # column-sum via an all-ones stationary matmul -> every psum
                # partition row holds the scores for 512 b (pre-broadcast).
                tva = pact1.tile([P, 9, b_c], f16, tag="tva")
                nc.vector.tensor_scalar_mul(tva[:], Tt[:], va_f[:, hh : hh + 1])
                attS = pact1.tile([P, 3, b_c], f16, tag="attS")
                for i in range(3):
                    # unnormalized softmax-weighted sum, single normalize at end
                    Ej = []
                    for j in range(3):
                        sc = ps_mm.tile([P, b_c], f32, tag="mm", name=f"sc{j}")
                        nc.tensor.matmul(
                            sc[:],
                            ones[:],
                            tva[:, 3 * i + j, :],
                            start=True,
                            stop=True,
                        )
                        e = psm.tile([P, b_c], f16, tag=f"E{j}")
                        nc.scalar.activation(e[:], sc[:], AF.Exp)
                        Ej.append(e)
                    den = psm.tile([P, b_c], f32, tag="den")
                    nc.vector.tensor_add(den[:], Ej[0][:], Ej[1][:])
                    nc.vector.tensor_add(den[:], den[:], Ej[2][:])
                    rden = psm.tile([P, b_c], f32, tag="rden")
                    nc.vector.reciprocal(rden[:], den[:])
                    acc = pact.tile([P, b_c], f32, tag="acc")
                    tmp = pact.tile([P, b_c], f32, tag="tmp")
                    nc.vector.tensor_mul(acc[:], vh[:, 0, :], Ej[0][:])
                    nc.vector.tensor_mul(tmp[:], vh[:, 1, :], Ej[1][:])
                    nc.vector.tensor_add(acc[:], acc[:], tmp[:])
                    nc.vector.tensor_mul(tmp[:], vh[:, 2, :], Ej[2][:])
                    nc.vector.tensor_add(acc[:], acc[:], tmp[:])
                    nc.vector.tensor_mul(attS[:, i, :], acc[:], rden[:])
                nc.sync.dma_start(attD[hh][:], attS[:])

            # ---- phase 5: output projection + residual ----
            # reuse tokT space for attT (same shape/layout, i = h*128 + d)
            for k in range(kt):
                nc.sync.dma_start(tokT[:, :, k, :], attD[k][:])
            for ot in range(kt):
                wT = load_wT(Wo, ot)
                for t in range(3):
                    po = ps_mm.tile([P, b_c], f32, tag="mm")
                    for k in range(kt):
                        nc.tensor.matmul(
                            po[:],
                            wT[:, k, :],
                            tokT[:, t, k, :],
                            start=(k == 0),
                            stop=(k == kt - 1),
                        )
                    # residual add in feature-major, then transpose out
                    origF = pact.tile([P, b_c], f16, tag="origF")
                    nc.sync.dma_start_transpose(
                        origF[:], tok_in[t][:, ot * P : (ot + 1) * P]
                    )
                    oTs = pact.tile([P, b_c], f32, tag="oTs")
                    nc.vector.tensor_add(oTs[:], po[:], origF[:])
                    obm3 = pact.tile([P, nb, P], f32, tag="obm3")
                    for bt in range(nb):
                        tp = ps_tp.tile([P, P], f32, tag="tp")
                        nc.tensor.transpose(
                            tp[:], oTs[:, bt * P : (bt + 1) * P], ident[:]
                        )
                        nc.any.tensor_copy(obm3[:, bt, :], tp[:])
                    nc.sync.dma_start(
                        outs[t][:, ot * P : (ot + 1) * P].rearrange(
                            "(bt p) o -> p bt o", p=P
                        ),
                        obm3[:],
                    )

    nc.compile()
    return nc


def _get_nc():
    key = "full"
    if key not in _compiled:
        _compiled[key] = _build()
    return _compiled[key]


def kernel(
    x_token,
    lat_token,
    fdbk_token,
    W_gate_L,
    b_gate_L,
    W_gate_X,
    b_gate_X,
    W_q,
    W_k,
    W_v,
    W_o,
    v_a,
):
    from concourse.bass_utils import run_bass_kernel_spmd

    nc = _get_nc()

    f32 = np.float32
    x2 = np.ascontiguousarray(np.asarray(x_token, f32).reshape(B, D).astype(np.float16))
    l2 = np.ascontiguousarray(np.asarray(lat_token, f32).reshape(B, D).astype(np.float16))
    f2 = np.ascontiguousarray(np.asarray(fdbk_token, f32).reshape(B, D).astype(np.float16))

    f16 = np.float16
    wgl = np.ascontiguousarray(np.asarray(W_gate_L, f32).astype(f16))
    wgx = np.ascontiguousarray(np.asarray(W_gate_X, f32).astype(f16))
    wq = np.ascontiguousarray(np.asarray(W_q, f32).astype(f16))
    wk = np.ascontiguousarray(np.asarray(W_k, f32).astype(f16))
    wv = np.ascontiguousarray(np.asarray(W_v, f32).astype(f16))
    wo = np.ascontiguousarray(np.asarray(W_o, f32).astype(f16))
    bglT = np.ascontiguousarray(np.asarray(b_gate_L, f32).reshape(KT, P).T)
    bgxT = np.ascontiguousarray(np.asarray(b_gate_X, f32).reshape(KT, P).T)
    vaT = np.ascontiguousarray(np.asarray(v_a, f32).reshape(H, DH).T)
    in_maps = []
    for c in range(N_CORES):
        s = slice(c * B_C, (c + 1) * B_C)
        in_maps.append(
            {
                "x": np.ascontiguousarray(x2[s]),
                "lat": np.ascontiguousarray(l2[s]),
                "fdbk": np.ascontiguousarray(f2[s]),
                "WgL": wgl,
                "WgX": wgx,
                "Wq": wq,
                "Wk": wk,
                "Wv": wv,
                "Wo": wo,
                "bgLT": bglT,
                "bgXT": bgxT,
                "vaT": vaT,
            }
        )

    res = run_bass_kernel_spmd(nc, in_maps, list(range(N_CORES))).results

    out = []
    for t in range(3):
        full = np.concatenate([res[c][f"o{t}"] for c in range(N_CORES)], axis=0)
        out.append(full.reshape(B, 1, D))
    return tuple(out)


# revision 13
# speedup vs baseline: 1.0595x; 1.0595x over previous
"""Trainium2 Bass kernel for nn_FLB_Attention_Layer (gated fusion + additive
attention over 3 tokens + output projection, with residuals).

Strategy: pure data-parallel over batch B=4096 across 8 NeuronCores
(512 samples/core, weights replicated). Inside each core:

- All activations are kept FEATURE-MAJOR in SBUF: [128 part = feature%128,
  k-tile = feature//128, token, batch]. Matmuls contract features on the
  partition dim (lhsT = W.T column block, rhs = activations).
- Weights are loaded row-contiguous [128 out-rows, 2048] and transposed
  on-chip via TensorE transpose-mode into W.T column blocks.
- Matmuls run in float32r (fp32 data, ~tf32 accuracy, bf16-rate at N>=512).
- Additive attention per head h (head dim 128 = one partition tile):
  T = tanh(q_i + k_j) built by DVE+ACT (bf16), scores via PE matmuls
  (lhsT = T slice, rhs = v_a column) -> batch-major scores [128b, 9],
  softmax batch-major on DVE/ACT, weights transposed back via PE and
  broadcast across partitions with a constant row-select matmul (SEL),
  weighted sum of v on DVE.
- Attention output (feature-major) roundtrips through a DRAM scratch,
  reusing the token SBUF space for the W_o contraction.
- Residual add done batch-major right before the final store.
"""

import numpy as np

P = 128
D = 2048
H = 16
DH = 128
KT = D // P  # 16 k-tiles
B = 4096
N_CORES = 8
B_C = B // N_CORES  # 512 per core

_compiled = {}


def _build(b_c=B_C, d=D, h=H):
    import concourse.bass as bass
    import concourse.mybir as mybir
    import concourse.tile as tile
    from contextlib import ExitStack
    from concourse import bacc
    from concourse.masks import make_identity

    f32 = mybir.dt.float32
    f32r = mybir.dt.float32r
    bf16 = mybir.dt.bfloat16
    AF = mybir.ActivationFunctionType

    kt = d // P
    nh = h
    nb = b_c // P  # batch tiles

    nc = bacc.Bacc(None, target_bir_lowering=False, debug=False)

    # ---- params ----
    f16 = mybir.dt.float16
    xs = nc.declare_dram_parameter("x", [b_c, d], f16, isOutput=False)
    ls = nc.declare_dram_parameter("lat", [b_c, d], f16, isOutput=False)
    fs = nc.declare_dram_parameter("fdbk", [b_c, d], f16, isOutput=False)
    WgL = nc.declare_dram_parameter("WgL", [d, d], f16, isOutput=False)
    WgX = nc.declare_dram_parameter("WgX", [d, d], f16, isOutput=False)
    Wq = nc.declare_dram_parameter("Wq", [d, d], f16, isOutput=False)
    Wk = nc.declare_dram_parameter("Wk", [d, d], f16, isOutput=False)
    Wv = nc.declare_dram_parameter("Wv", [d, d], f16, isOutput=False)
    Wo = nc.declare_dram_parameter("Wo", [d, d], f16, isOutput=False)
    bgLT = nc.declare_dram_parameter("bgLT", [P, kt], f32, isOutput=False)
    bgXT = nc.declare_dram_parameter("bgXT", [P, kt], f32, isOutput=False)
    vaT = nc.declare_dram_parameter("vaT", [DH, nh], f32, isOutput=False)

    outs = [
        nc.declare_dram_parameter(f"o{t}", [b_c, d], f32, isOutput=True)
        for t in range(3)
    ]
    tok_in = [xs, ls, fs]

    with tile.TileContext(nc) as tc:
        with ExitStack() as ctx:
            const = ctx.enter_context(tc.tile_pool(name="const", bufs=1))
            ptok = ctx.enter_context(tc.tile_pool(name="ptok", bufs=1))
            pbig = ctx.enter_context(tc.tile_pool(name="pbig", bufs=2))
            pwT = ctx.enter_context(tc.tile_pool(name="pwT", bufs=4))
            pact = ctx.enter_context(tc.tile_pool(name="pact", bufs=2))
            pact1 = ctx.enter_context(tc.tile_pool(name="pact1", bufs=2))
            pvh = ctx.enter_context(tc.tile_pool(name="pvh", bufs=2))
            psm = ctx.enter_context(tc.tile_pool(name="psm", bufs=2))
            dram = ctx.enter_context(tc.tile_pool(name="dram", bufs=1, space="DRAM"))
            ps_mm = ctx.enter_context(tc.tile_pool(name="ps_mm", bufs=6, space="PSUM"))
            ps_tp = ctx.enter_context(tc.tile_pool(name="ps_tp", bufs=2, space="PSUM"))

            ident = const.tile([P, P], f32)
            make_identity(nc, ident)
            bgl_t = const.tile([P, kt], f32)
            bgx_t = const.tile([P, kt], f32)
            nc.sync.dma_start(bgl_t[:], bgLT[:])
            nc.sync.dma_start(bgx_t[:], bgXT[:])
            va_f = const.tile([DH, nh], f32)
            nc.sync.dma_start(va_f[:], vaT[:])
            ones = const.tile([P, P], f16)
            nc.any.memset(ones[:], 1.0)

            # tokT: feature-major tokens [p, k, tok, b]; later reused for attT
            tokT = ptok.tile([P, 3, kt, b_c], f16)

            def transpose_block(src_ap, dst_ap):
                """PE-transpose a [128, 128] block; evict (cast) on DVE."""
                tp = ps_tp.tile([P, P], f32, tag="tp")
                nc.tensor.transpose(tp[:], src_ap, ident[:])
                nc.vector.tensor_copy(dst_ap, tp[:])

            # ---- phase 1: XBAR-transposed token loads (feature-major) ----
            origFM = dram.tile([P, 3, kt, b_c], f16)
            for t in (2, 1, 0):
                nc.sync.dma_start_transpose(tokT[:, t, :, :], tok_in[t][:])
                # stash pristine feature-major token for the phase-5 residual
                nc.sync.dma_start(origFM[:, t, :, :], tokT[:, t, :, :])

            def load_wT(Wt, row_tile):
                """XBAR-transposed load of fp16 weight rows [128, d] into the
                W.T block [128 (in-feature part), kt, 128 (out cols)]."""
                wT = pwT.tile([P, kt, P], f16, tag="wT")
                nc.sync.dma_start_transpose(
                    wT[:], Wt[row_tile * P : (row_tile + 1) * P, :]
                )
                return wT

            # ---- phases 2+3: gated fusion ----
            # G_L = sigmoid(fdbk @ WgL.T + bgL); lat *= G_L
            # G_X = sigmoid(lat' @ WgX.T + bgX); x *= G_X
            for stage, (Wg, bg_t, src_tok, dst_tok) in enumerate(
                [(WgL, bgl_t, 2, 1), (WgX, bgx_t, 1, 0)]
            ):
                for ot in range(kt):
                    wT = load_wT(Wg, ot)
                    pg = ps_mm.tile([P, b_c], f32, tag="mm")
                    for k in range(kt):
                        nc.tensor.matmul(
                            pg[:],
                            wT[:, k, :],
                            tokT[:, src_tok, k, :],
                            start=(k == 0),
                            stop=(k == kt - 1),
                        )
                    gate = pact.tile([P, b_c], f32, tag="gate")
                    nc.scalar.activation(
                        gate[:], pg[:], AF.Sigmoid, bias=bg_t[:, ot : ot + 1]
                    )
                    nc.vector.tensor_mul(
                        tokT[:, dst_tok, ot, :],
                        tokT[:, dst_tok, ot, :],
                        gate[:],
                    )

            # ---- phase 4: per-head QKV + additive attention ----
            attD = [dram.tile([P, 3, b_c], f16, name=f"attD{i}") for i in range(nh)]
            for hh in range(nh):
                qkv_sb = []
                for Wp in (Wq, Wk, Wv):
                    wT = load_wT(Wp, hh)
                    pool_p = pvh if len(qkv_sb) == 2 else pact1
                    dst = pool_p.tile([P, 3, b_c], f16, tag=f"p{len(qkv_sb)}")
                    pps = [ps_mm.tile([P, b_c], f32, tag="mm", name=f"pp{t}") for t in range(3)]
                    for k in range(kt):
                        for t in range(3):
                            nc.tensor.matmul(
                                pps[t][:],
                                wT[:, k, :],
                                tokT[:, t, k, :],
                                start=(k == 0),
                                stop=(k == kt - 1),
                            )
                    for t in range(3):
                        nc.any.tensor_copy(dst[:, t, :], pps[t][:])
                    qkv_sb.append(dst)
                qh, kh, vh = qkv_sb

                # T = tanh(q_i + k_j), bf16 [p, ij, b]
                Tt = pact1.tile([P, 9, b_c], f16, tag="Tt")
                for i in range(3):
                    for j in range(3):
                        pre = pact.tile([P, b_c], f16, tag="Tpre")
                        nc.vector.tensor_add(pre[:], qh[:, i, :], kh[:, j, :])
                        nc.scalar.activation(Tt[:, 3 * i + j, :], pre[:], AF.Tanh)

                # scores: tva = Tt * va_h (per-partition scalar), then
                # column-sum via an all-ones stationary matmul -> every psum
                # partition row holds the scores for 512 b (pre-broadcast).
                tva = pact1.tile([P, 9, b_c], f16, tag="tva")
                nc.vector.tensor_scalar_mul(tva[:], Tt[:], va_f[:, hh : hh + 1])
                attS = pact1.tile([P, 3, b_c], f16, tag="attS")
                for i in range(3):
                    # unnormalized softmax-weighted sum, single normalize at end
                    Ej = []
                    for j in range(3):
                        sc = ps_mm.tile([P, b_c], f32, tag="mm", name=f"sc{j}")
                        nc.tensor.matmul(
                            sc[:],
                            ones[:],
                            tva[:, 3 * i + j, :],
                            start=True,
                            stop=True,
                        )
                        e = psm.tile([P, b_c], f16, tag=f"E{j}")
                        nc.scalar.activation(e[:], sc[:], AF.Exp)
                        Ej.append(e)
                    den = psm.tile([P, b_c], f32, tag="den")
                    nc.vector.tensor_add(den[:], Ej[0][:], Ej[1][:])
                    nc.vector.tensor_add(den[:], den[:], Ej[2][:])
                    rden = psm.tile([P, b_c], f32, tag="rden")
                    nc.vector.reciprocal(rden[:], den[:])
                    acc = pact.tile([P, b_c], f32, tag="acc")
                    tmp = pact.tile([P, b_c], f32, tag="tmp")
                    nc.vector.tensor_mul(acc[:], vh[:, 0, :], Ej[0][:])
                    nc.vector.tensor_mul(tmp[:], vh[:, 1, :], Ej[1][:])
                    nc.vector.tensor_add(acc[:], acc[:], tmp[:])
                    nc.vector.tensor_mul(tmp[:], vh[:, 2, :], Ej[2][:])
                    nc.vector.tensor_add(acc[:], acc[:], tmp[:])
                    nc.vector.tensor_mul(attS[:, i, :], acc[:], rden[:])
                nc.sync.dma_start(attD[hh][:], attS[:])

            # ---- phase 5: output projection + residual ----
            # reuse tokT space for attT (same shape/layout, i = h*128 + d)
            for k in range(kt):
                nc.sync.dma_start(tokT[:, :, k, :], attD[k][:])
            for ot in range(kt):
                wT = load_wT(Wo, ot)
                for t in range(3):
                    po = ps_mm.tile([P, b_c], f32, tag="mm")
                    for k in range(kt):
                        nc.tensor.matmul(
                            po[:],
                            wT[:, k, :],
                            tokT[:, t, k, :],
                            start=(k == 0),
                            stop=(k == kt - 1),
                        )
                    # residual add in feature-major, then transpose out
                    origF = pact.tile([P, b_c], f16, tag="origF")
                    nc.sync.dma_start(origF[:], origFM[:, t, ot, :])
                    oTs = pact.tile([P, b_c], f32, tag="oTs")
                    nc.vector.tensor_add(oTs[:], po[:], origF[:])
                    obm3 = pact.tile([P, nb, P], f32, tag="obm3")
                    for bt in range(nb):
                        tp = ps_tp.tile([P, P], f32, tag="tp")
                        nc.tensor.transpose(
                            tp[:], oTs[:, bt * P : (bt + 1) * P], ident[:]
                        )
                        nc.any.tensor_copy(obm3[:, bt, :], tp[:])
                    nc.sync.dma_start(
                        outs[t][:, ot * P : (ot + 1) * P].rearrange(
                            "(bt p) o -> p bt o", p=P
                        ),
                        obm3[:],
                    )

    nc.compile()
    return nc


def _get_nc():
    key = "full"
    if key not in _compiled:
        _compiled[key] = _build()
    return _compiled[key]


def kernel(
    x_token,
    lat_token,
    fdbk_token,
    W_gate_L,
    b_gate_L,
    W_gate_X,
    b_gate_X,
    W_q,
    W_k,
    W_v,
    W_o,
    v_a,
):
    from concourse.bass_utils import run_bass_kernel_spmd

    nc = _get_nc()

    f32 = np.float32
    x2 = np.ascontiguousarray(np.asarray(x_token, f32).reshape(B, D).astype(np.float16))
    l2 = np.ascontiguousarray(np.asarray(lat_token, f32).reshape(B, D).astype(np.float16))
    f2 = np.ascontiguousarray(np.asarray(fdbk_token, f32).reshape(B, D).astype(np.float16))

    f16 = np.float16
    wgl = np.ascontiguousarray(np.asarray(W_gate_L, f32).astype(f16))
    wgx = np.ascontiguousarray(np.asarray(W_gate_X, f32).astype(f16))
    wq = np.ascontiguousarray(np.asarray(W_q, f32).astype(f16))
    wk = np.ascontiguousarray(np.asarray(W_k, f32).astype(f16))
    wv = np.ascontiguousarray(np.asarray(W_v, f32).astype(f16))
    wo = np.ascontiguousarray(np.asarray(W_o, f32).astype(f16))
    bglT = np.ascontiguousarray(np.asarray(b_gate_L, f32).reshape(KT, P).T)
    bgxT = np.ascontiguousarray(np.asarray(b_gate_X, f32).reshape(KT, P).T)
    vaT = np.ascontiguousarray(np.asarray(v_a, f32).reshape(H, DH).T)
    in_maps = []
    for c in range(N_CORES):
        s = slice(c * B_C, (c + 1) * B_C)
        in_maps.append(
            {
                "x": np.ascontiguousarray(x2[s]),
                "lat": np.ascontiguousarray(l2[s]),
                "fdbk": np.ascontiguousarray(f2[s]),
                "WgL": wgl,
                "WgX": wgx,
                "Wq": wq,
                "Wk": wk,
                "Wv": wv,
                "Wo": wo,
                "bgLT": bglT,
                "bgXT": bgxT,
                "vaT": vaT,
            }
        )

    res = run_bass_kernel_spmd(nc, in_maps, list(range(N_CORES))).results

    out = []
    for t in range(3):
        full = np.concatenate([res[c][f"o{t}"] for c in range(N_CORES)], axis=0)
        out.append(full.reshape(B, 1, D))
    return tuple(out)
